# revision 20
# baseline (speedup 1.0000x reference)
"""Additive (Bahdanau) attention as a TRN2 Bass/Tile kernel, SPMD over 8 cores.

Math per batch b (shapes: Q (256,256), K (1024,256), V (1024,256), H=128):
    qp = Q @ Wq.T                       (NQ, H)
    kp = K @ Wk.T                       (NKV, H)
    s[i, j]  = sum_h Wv[h] * tanh(qp[i, h] + kp[j, h])
    attn     = masked softmax_j(s)      (j < valid_lens[b])
    out      = attn @ V                 (NQ, DV)

Device decomposition: work is split into "slots" of 128 contiguous keys of one
batch.  Each slot computes the *unnormalized* softmax partials over its keys
for all 256 queries:

    num[i, :] = sum_j exp(s[i, j]) * V[j, :]      den[i] = sum_j exp(s[i, j])

The host sums the partials per batch and divides.  exp is applied without
max-subtraction (|s| <= sum_h |Wv[h]|, a few units, so exp cannot overflow),
which makes the partial sums mathematically exact under any key split.  That
lets the host:
  * skip key blocks that are entirely masked (j >= valid_lens[b]),
  * load-balance the surviving slots evenly across the 8 cores.
Masked keys inside a boundary slot contribute nothing because the host zeroes
their rows of [V | 1] (both the numerator rows and the ones column).

Per-slot device pipeline (H=128 lives on the SBUF partition axis):
  PE    kpT(128h,128j) / qpT(128h,256i) projections from host-pre-transposed
        K/Q chunks (layout prep only; the FLOPs happen on device)
  DVE   sums[h, jj, i] = qpT[h, i] + kpT[h, j]     (tensor_scalar_add per key)
  ACT   tanh over a whole 16-key group in one instruction (128 x 4096)
  PE    per key: stationary (Wv o+ I32) column -> accumulates score rows into
        a 32-aligned (32, 256) PSUM block (documented col-tiling pattern)
  ACT   exp over the slot's scoresT (128j, 256i), PSUM -> SBUF
  PE    expT.T @ [V | 1]  -> (128i, VE_W) partials, 2 query chunks
  DMA   PSUM -> DRAM
"""

import os
from contextlib import ExitStack

import numpy as np

B, NQ, NKV, D, H = 8, 256, 1024, 256, 128
NCORES = 8
SLOT_KEYS = 128          # keys per slot
ACT_G = 32               # keys per tanh group (one ACT instruction each)
VE_W = 264               # 256 V cols + 1 ones col + 7 pad cols
DEN_COL = 256            # index of the denominator column in VE / out

_prog_cache: dict[int, object] = {}


def _build_program(cap: int):
    """Build + compile the Bass program for `cap` slots per core."""
    import concourse.bass as bass  # noqa: F401  (registers engines)
    import concourse.tile as tile
    from concourse import bacc, mybir

    f32 = mybir.dt.float32
    bf16 = mybir.dt.bfloat16
    AF = mybir.ActivationFunctionType

    nc = bacc.Bacc("TRN2", target_bir_lowering=False, debug=False,
                   num_devices=NCORES)

    # DRAM I/O.  Layouts chosen so every DMA is contiguous per partition.
    kt = nc.dram_tensor("kt", [cap, 128, 2, 128], f32, kind="ExternalInput")
    qt = nc.dram_tensor("qt", [cap, 128, 2, 256], f32, kind="ExternalInput")
    ve = nc.dram_tensor("ve", [cap, 128, VE_W], f32, kind="ExternalInput")
    wqt = nc.dram_tensor("wqt", [128, 2, 128], f32, kind="ExternalInput")
    wkt = nc.dram_tensor("wkt", [128, 2, 128], f32, kind="ExternalInput")
    wvd = nc.dram_tensor("wvd", [128, 32, 32], bf16, kind="ExternalInput")
    out = nc.dram_tensor("out", [cap, 2, 128, VE_W], f32, kind="ExternalOutput")

    # Per-slot key schedule: 3 groups of 32 + 1 group of 24 built by DVE
    # adds + grouped tanh; the last 8 keys use ACT's fused bias path
    # (tanh(qp + kp_j) in one ACTIVATE) to keep DVE and ACT balanced.
    BIAS_KEYS = 8
    groups = [(0, 32), (32, 32), (64, 32), (96, SLOT_KEYS - 96 - BIAS_KEYS)]

    with tile.TileContext(nc) as tc:
        with ExitStack() as ctx:
            consts = ctx.enter_context(tc.tile_pool(name="consts", bufs=1))
            kin = ctx.enter_context(tc.tile_pool(name="kin", bufs=2))
            qin = ctx.enter_context(tc.tile_pool(name="qin", bufs=2))
            vin = ctx.enter_context(tc.tile_pool(name="vin", bufs=2))
            proj = ctx.enter_context(tc.tile_pool(name="proj", bufs=2))
            sums_p = ctx.enter_context(tc.tile_pool(name="sums", bufs=2))
            tanh_p = ctx.enter_context(tc.tile_pool(name="tanh", bufs=2))
            exp_p = ctx.enter_context(tc.tile_pool(name="expp", bufs=2))
            ps_proj = ctx.enter_context(
                tc.tile_pool(name="psproj", bufs=2, space="PSUM"))
            ps_sc = ctx.enter_context(
                tc.tile_pool(name="pssc", bufs=2, space="PSUM"))
            ps_out = ctx.enter_context(
                tc.tile_pool(name="psout", bufs=2, space="PSUM"))

            wqt_sb = consts.tile([128, 2, 128], f32)
            nc.sync.dma_start(out=wqt_sb[:], in_=wqt[:])
            wkt_sb = consts.tile([128, 2, 128], f32)
            nc.sync.dma_start(out=wkt_sb[:], in_=wkt[:])
            wvd_sb = consts.tile([128, 32, 32], bf16)
            nc.sync.dma_start(out=wvd_sb[:], in_=wvd[:])

            def load_and_project(s):
                """DMA slot s inputs + compute kpT/qpT; returns SBUF tiles."""
                kt_sb = kin.tile([128, 2, 128], f32, tag="kt")
                nc.sync.dma_start(out=kt_sb[:], in_=kt[s])
                qt_sb = qin.tile([128, 2, 256], f32, tag="qt")
                nc.sync.dma_start(out=qt_sb[:], in_=qt[s])
                ve_sb = vin.tile([128, VE_W], f32, tag="ve")
                nc.sync.dma_start(out=ve_sb[:], in_=ve[s])

                # kpT[h, j] = sum_d Wk[h, d] K[j, d]  (contract d on partitions)
                kp_ps = ps_proj.tile([128, 128], f32, tag="kp")
                for c in range(2):
                    nc.tensor.matmul(kp_ps[:], wkt_sb[:, c, :], kt_sb[:, c, :],
                                     start=(c == 0), stop=(c == 1))
                kp_sb = proj.tile([128, 128], f32, tag="kp_sb")
                nc.scalar.copy(kp_sb[:], kp_ps[:])

                qp_ps = ps_proj.tile([128, 256], f32, tag="qp")
                for c in range(2):
                    nc.tensor.matmul(qp_ps[:], wqt_sb[:, c, :], qt_sb[:, c, :],
                                     start=(c == 0), stop=(c == 1))
                qp_sb = proj.tile([128, 256], bf16, tag="qp_sb")
                nc.scalar.copy(qp_sb[:], qp_ps[:])
                return kp_sb, qp_sb, ve_sb

            nxt = load_and_project(0)
            for s in range(cap):
                kp_sb, qp_sb, ve_sb = nxt
                if s + 1 < cap:
                    # software-pipeline: next slot's loads + projections are
                    # emitted first so each engine's FIFO has them before
                    # this slot's long tanh/add streams
                    nxt = load_and_project(s + 1)

                # scoresT[j, i] for this slot, built 32 rows at a time.
                sc_ps = ps_sc.tile([128, 256], f32, tag="sc")

                def score_mm(j, rhs):
                    sg, jl = divmod(j, 32)
                    nc.tensor.matmul(
                        sc_ps[sg * 32:(sg + 1) * 32, :],
                        wvd_sb[:, jl, :],
                        rhs,
                        start=(jl == 0), stop=(jl == 31),
                        tile_position=(0, sg * 32))

                for j0, glen in groups:
                    sums = sums_p.tile([128, ACT_G, 256], bf16, tag="sums")
                    for jj in range(glen):
                        nc.vector.tensor_scalar_add(
                            out=sums[:, jj, :], in0=qp_sb[:],
                            scalar1=kp_sb[:, j0 + jj:j0 + jj + 1])
                    th = tanh_p.tile([128, ACT_G, 256], bf16, tag="th")
                    nc.scalar.activation(out=th[:, :glen, :],
                                         in_=sums[:, :glen, :], func=AF.Tanh)
                    for jj in range(glen):
                        score_mm(j0 + jj, th[:, jj, :])

                # tail keys: fused tanh(qp + kp_j) on ACT, no DVE add
                thb = tanh_p.tile([128, BIAS_KEYS, 256], bf16, tag="thb")
                for bk in range(BIAS_KEYS):
                    j = SLOT_KEYS - BIAS_KEYS + bk
                    nc.scalar.activation(out=thb[:, bk, :], in_=qp_sb[:],
                                         func=AF.Tanh,
                                         bias=kp_sb[:, j:j + 1])
                    score_mm(j, thb[:, bk, :])

                exp_sb = exp_p.tile([128, 256], f32, tag="exp")
                nc.scalar.activation(out=exp_sb[:], in_=sc_ps[:], func=AF.Exp)

                for ic in range(2):
                    o_ps = ps_out.tile([128, VE_W], f32, tag="o")
                    nc.tensor.matmul(o_ps[:],
                                     exp_sb[:, ic * 128:(ic + 1) * 128],
                                     ve_sb[:],
                                     start=True, stop=True)
                    o_sb = exp_p.tile([128, VE_W], f32, tag="o_sb")
                    nc.scalar.copy(o_sb[:], o_ps[:])
                    nc.sync.dma_start(out=out[s, ic], in_=o_sb[:])

    nc.compile()
    return nc


def _get_program(cap: int):
    if cap not in _prog_cache:
        _prog_cache[cap] = _build_program(cap)
    return _prog_cache[cap]


def _chunkT(a2d: np.ndarray, nfree: int) -> np.ndarray:
    """(n, 256) row-major -> (128, 2, n): [p, c, n] = a2d[n, 128c + p]."""
    return np.ascontiguousarray(
        a2d.T.reshape(2, 128, nfree).transpose(1, 0, 2))


def _prepare(Q_batch, K_batch, V_batch, valid_lens, Wq, Wk, Wv):
    Q = np.asarray(Q_batch, np.float32)
    K = np.asarray(K_batch, np.float32)
    V = np.asarray(V_batch, np.float32)
    L = np.asarray(valid_lens).astype(np.int64)
    Wq = np.asarray(Wq, np.float32)
    Wk = np.asarray(Wk, np.float32)
    Wv = np.asarray(Wv, np.float32)

    # Work list: one slot per 128-key block that contains any valid key.
    slots = []
    for b in range(B):
        nblk = max(1, int(-(-int(L[b]) // SLOT_KEYS)))
        nblk = min(nblk, NKV // SLOT_KEYS)
        for blk in range(nblk):
            slots.append((b, blk * SLOT_KEYS))
    cap = -(-len(slots) // NCORES)

    import ml_dtypes
    wqt = _chunkT(Wq, 128)
    wkt = _chunkT(Wk, 128)
    wvd = np.zeros((128, 32, 32), np.float32)
    wvd[:, np.arange(32), np.arange(32)] = Wv[:, None]
    wvd = wvd.astype(ml_dtypes.bfloat16)

    qts = [_chunkT(Q[b], 256) for b in range(B)]

    in_maps = []
    core_slots = []
    for c in range(NCORES):
        items = slots[c * cap:(c + 1) * cap]
        core_slots.append(items)
        kt_arr = np.zeros((cap, 128, 2, 128), np.float32)
        qt_arr = np.zeros((cap, 128, 2, 256), np.float32)
        ve_arr = np.zeros((cap, 128, VE_W), np.float32)
        for si, (b, j0) in enumerate(items):
            kt_arr[si] = _chunkT(K[b, j0:j0 + SLOT_KEYS], SLOT_KEYS)
            qt_arr[si] = qts[b]
            nval = int(np.clip(int(L[b]) - j0, 0, SLOT_KEYS))
            ve_arr[si, :nval, :256] = V[b, j0:j0 + nval]
            ve_arr[si, :nval, DEN_COL] = 1.0
        in_maps.append({
            "kt": kt_arr, "qt": qt_arr, "ve": ve_arr,
            "wqt": wqt, "wkt": wkt, "wvd": wvd,
        })
    return cap, core_slots, in_maps


def _gather(core_slots, results) -> np.ndarray:
    acc = np.zeros((B, NQ, 257), np.float64)
    for c, items in enumerate(core_slots):
        o = results[c]["out"]  # (cap, 2, 128, VE_W)
        for si, (b, _j0) in enumerate(items):
            part = o[si].reshape(NQ, VE_W)[:, :257]
            acc[b] += part
    return (acc[:, :, :256] / acc[:, :, 256:257]).astype(np.float32)


def _install_ntff_hook():
    """Register the axon NTFF profile hook that bass_utils reads via
    antenv.axon_hooks (the shipped antenv stub lacks that module)."""
    import contextlib
    import ctypes
    import sys
    import types

    try:
        from antenv.axon_hooks import get_axon_ntff_profile_hook
        if get_axon_ntff_profile_hook() is not None:
            return
    except ImportError:
        pass

    so_path = "/opt/axon/libaxon_pjrt.so"
    if not os.path.exists(so_path):
        return
    lib = ctypes.CDLL(so_path)
    if not hasattr(lib, "axon_start_nrt_profile"):
        return
    lib.axon_start_nrt_profile.argtypes = [
        ctypes.POINTER(ctypes.c_int64), ctypes.c_size_t]
    lib.axon_start_nrt_profile.restype = ctypes.c_int64
    lib.axon_stop_nrt_profile.argtypes = [ctypes.c_char_p]
    lib.axon_stop_nrt_profile.restype = ctypes.c_int64

    @contextlib.contextmanager
    def _hook(output_dir, device_ids):
        import jax
        jax.devices()
        if device_ids:
            ids = (ctypes.c_int64 * len(device_ids))(*device_ids)
            rc = lib.axon_start_nrt_profile(ids, len(device_ids))
        else:
            rc = lib.axon_start_nrt_profile(None, 0)
        if rc != 0:
            raise RuntimeError(f"axon_start_nrt_profile rc={rc}")
        try:
            yield
        finally:
            n = lib.axon_stop_nrt_profile(str(output_dir).encode())
            print(f"ntff profile: {n} file(s) written to {output_dir}")

    mod = types.ModuleType("antenv.axon_hooks")
    mod.get_axon_ntff_profile_hook = lambda: _hook
    mod.set_axon_ntff_profile_hook = lambda h: None
    sys.modules["antenv.axon_hooks"] = mod
    import antenv
    antenv.axon_hooks = mod


def run(Q_batch, K_batch, V_batch, valid_lens, Wq, Wk, Wv,
        trace: bool = False):
    """Returns (output, exec_time_ns_or_None)."""
    from concourse.bass_utils import run_bass_kernel_spmd

    if trace:
        _install_ntff_hook()

    cap, core_slots, in_maps = _prepare(
        Q_batch, K_batch, V_batch, valid_lens, Wq, Wk, Wv)
    nc = _get_program(cap)

    if os.environ.get("ADD_ATTN_SIM"):
        from concourse.bass_interp import CoreSim
        ncores = int(os.environ.get("ADD_ATTN_SIM_CORES", NCORES))
        results = []
        for c in range(ncores):
            sim = CoreSim(nc)
            for name, arr in in_maps[c].items():
                sim.tensor(name)[:] = arr
            sim.simulate()
            results.append({"out": np.array(sim.tensor("out"))})
        core_slots = core_slots[:ncores]
        return _gather(core_slots, results), None

    res = run_bass_kernel_spmd(nc, in_maps, core_ids=list(range(NCORES)),
                               trace=trace)
    return _gather(core_slots, res.results), res.exec_time_ns


def kernel(Q_batch, K_batch, V_batch, valid_lens, Wq, Wk, Wv):
    out, _ = run(Q_batch, K_batch, V_batch, valid_lens, Wq, Wk, Wv)
    return out


# revision 28
# speedup vs baseline: 1.2014x; 1.2014x over previous
"""Additive (Bahdanau) attention as a TRN2 Bass/Tile kernel, SPMD over 8 cores.

Math per batch b (shapes: Q (256,256), K (1024,256), V (1024,256), H=128):
    qp = Q @ Wq.T                       (NQ, H)
    kp = K @ Wk.T                       (NKV, H)
    s[i, j]  = sum_h Wv[h] * tanh(qp[i, h] + kp[j, h])
    attn     = masked softmax_j(s)      (j < valid_lens[b])
    out      = attn @ V                 (NQ, DV)

Device decomposition: work is split into "slots" of 128 contiguous keys of one
batch.  Each slot computes the *unnormalized* softmax partials over its keys
for all 256 queries:

    num[i, :] = sum_j exp(s[i, j]) * V[j, :]      den[i] = sum_j exp(s[i, j])

The host sums the partials per batch and divides.  exp is applied without
max-subtraction (|s| <= sum_h |Wv[h]|, a few units, so exp cannot overflow),
which makes the partial sums mathematically exact under any key split.  That
lets the host:
  * skip key blocks that are entirely masked (j >= valid_lens[b]),
  * load-balance the surviving slots evenly across the 8 cores.
Masked keys inside a boundary slot contribute nothing because the host zeroes
their rows of [V | 1] (both the numerator rows and the ones column).

Per-slot device pipeline (H=128 lives on the SBUF partition axis):
  PE    kpT(128h,128j) / qpT(128h,256i) projections from host-pre-transposed
        K/Q chunks (layout prep only; the FLOPs happen on device)
  DVE   sums[h, jj, i] = qpT[h, i] + kpT[h, j]  in bf16
        (one tensor_scalar_add per key; ~277 ns/op is the kernel's wall)
  ACT   tanh over a whole 32-key group in one instruction (128 x 8192 bf16)
  PE    per key: stationary (Wv o+ I32) column, bf16 -> accumulates score
        rows into a 32-aligned (32, 256) fp32 PSUM block (col-tiling)
  ACT   exp over the slot's scoresT (128j, 256i), PSUM -> SBUF fp32
  PE    expT.T @ [V | 1] fp32 -> (128i, VE_W) partials, 2 query chunks
  DMA   PSUM -> SBUF (DVE copy) -> DRAM

Measured on TRN2 (axon), seed-0 inputs (cap=4): ~147.5 us HW exec,
rel err ~8e-4 vs the fp32 jax reference (bf16 tanh path dominates the
error).  Dense worst case (all lens 1023, cap=8): ~265 us.
"""

import os
from contextlib import ExitStack

import numpy as np

B, NQ, NKV, D, H = 8, 256, 1024, 256, 128
NCORES = 8
SLOT_KEYS = 128          # keys per slot
ACT_G = 32               # max keys per tanh group (one ACT instruction each)
VE_W = 264               # 256 V cols + 1 ones col + 7 pad cols
DEN_COL = 256            # index of the denominator column in VE / out

_prog_cache: dict[tuple, object] = {}

# kernel structure knobs (tuned on HW 2026-08-03; ~147.5us at cap=4)
CONFIG = {
    "bias_keys": 0,       # keys per slot via ACT fused-bias tanh (no DVE add)
    "copies": "dve",      # engine for PSUM->SBUF copies: "act" | "dve"
    "prefetch": True,     # emit slot s+1 loads/projections before slot s body
    "sums_bufs": 4,
    "th_bufs": 4,
    "act_g": 32,          # keys per grouped-tanh ACT instruction
}


def _build_program(cap: int):
    """Build + compile the Bass program for `cap` slots per core."""
    import concourse.bass as bass  # noqa: F401  (registers engines)
    import concourse.tile as tile
    from concourse import bacc, mybir

    f32 = mybir.dt.float32
    bf16 = mybir.dt.bfloat16
    AF = mybir.ActivationFunctionType

    nc = bacc.Bacc("TRN2", target_bir_lowering=False, debug=False,
                   num_devices=NCORES)

    # DRAM I/O.  Layouts chosen so every DMA is contiguous per partition.
    kt = nc.dram_tensor("kt", [cap, 128, 2, 128], f32, kind="ExternalInput")
    qt = nc.dram_tensor("qt", [cap, 128, 2, 256], f32, kind="ExternalInput")
    ve = nc.dram_tensor("ve", [cap, 128, VE_W], f32, kind="ExternalInput")
    wqt = nc.dram_tensor("wqt", [128, 2, 128], f32, kind="ExternalInput")
    wkt = nc.dram_tensor("wkt", [128, 2, 128], f32, kind="ExternalInput")
    wvd = nc.dram_tensor("wvd", [128, 32, 32], bf16, kind="ExternalInput")
    out = nc.dram_tensor("out", [cap, 2, 128, VE_W], f32, kind="ExternalOutput")

    # Per-slot key schedule: groups of <=32 keys built by DVE adds + one
    # grouped tanh each; optionally the last BIAS_KEYS keys use ACT's fused
    # bias path (tanh(qp + kp_j) in one ACTIVATE, no DVE add) to balance
    # DVE and ACT.
    BIAS_KEYS = CONFIG["bias_keys"]
    gsz = CONFIG["act_g"]
    ndve = SLOT_KEYS - BIAS_KEYS
    groups = []
    j0 = 0
    while j0 < ndve:
        groups.append((j0, min(gsz, ndve - j0)))
        j0 += gsz

    with tile.TileContext(nc) as tc:
        with ExitStack() as ctx:
            consts = ctx.enter_context(tc.tile_pool(name="consts", bufs=1))
            kin = ctx.enter_context(tc.tile_pool(name="kin", bufs=2))
            qin = ctx.enter_context(tc.tile_pool(name="qin", bufs=2))
            vin = ctx.enter_context(tc.tile_pool(name="vin", bufs=2))
            proj = ctx.enter_context(tc.tile_pool(name="proj", bufs=2))
            sums_p = ctx.enter_context(
                tc.tile_pool(name="sums", bufs=CONFIG["sums_bufs"]))
            tanh_p = ctx.enter_context(
                tc.tile_pool(name="tanh", bufs=CONFIG["th_bufs"]))
            exp_p = ctx.enter_context(tc.tile_pool(name="expp", bufs=2))
            ps_proj = ctx.enter_context(
                tc.tile_pool(name="psproj", bufs=2, space="PSUM"))
            ps_sc = ctx.enter_context(
                tc.tile_pool(name="pssc", bufs=2, space="PSUM"))
            ps_out = ctx.enter_context(
                tc.tile_pool(name="psout", bufs=2, space="PSUM"))

            wqt_sb = consts.tile([128, 2, 128], f32)
            nc.sync.dma_start(out=wqt_sb[:], in_=wqt[:])
            wkt_sb = consts.tile([128, 2, 128], f32)
            nc.sync.dma_start(out=wkt_sb[:], in_=wkt[:])
            wvd_sb = consts.tile([128, 32, 32], bf16)
            nc.sync.dma_start(out=wvd_sb[:], in_=wvd[:])

            copy_eng = (nc.scalar.copy if CONFIG["copies"] == "act"
                        else nc.vector.tensor_copy)

            def load_and_project(s):
                """DMA slot s inputs + compute kpT/qpT; returns SBUF tiles."""
                kt_sb = kin.tile([128, 2, 128], f32, tag="kt")
                nc.sync.dma_start(out=kt_sb[:], in_=kt[s])
                qt_sb = qin.tile([128, 2, 256], f32, tag="qt")
                nc.sync.dma_start(out=qt_sb[:], in_=qt[s])
                ve_sb = vin.tile([128, VE_W], f32, tag="ve")
                nc.sync.dma_start(out=ve_sb[:], in_=ve[s])

                # kpT[h, j] = sum_d Wk[h, d] K[j, d]  (contract d on partitions)
                kp_ps = ps_proj.tile([128, 128], f32, tag="kp")
                for c in range(2):
                    nc.tensor.matmul(kp_ps[:], wkt_sb[:, c, :], kt_sb[:, c, :],
                                     start=(c == 0), stop=(c == 1))
                kp_sb = proj.tile([128, 128], f32, tag="kp_sb")
                copy_eng(kp_sb[:], kp_ps[:])

                qp_ps = ps_proj.tile([128, 256], f32, tag="qp")
                for c in range(2):
                    nc.tensor.matmul(qp_ps[:], wqt_sb[:, c, :], qt_sb[:, c, :],
                                     start=(c == 0), stop=(c == 1))
                qp_sb = proj.tile([128, 256], bf16, tag="qp_sb")
                copy_eng(qp_sb[:], qp_ps[:])
                return kp_sb, qp_sb, ve_sb

            nxt = load_and_project(0)
            for s in range(cap):
                if not CONFIG["prefetch"] and s > 0:
                    nxt = load_and_project(s)
                kp_sb, qp_sb, ve_sb = nxt
                if CONFIG["prefetch"] and s + 1 < cap:
                    # software-pipeline: next slot's loads + projections are
                    # emitted first so each engine's FIFO has them before
                    # this slot's long tanh/add streams
                    nxt = load_and_project(s + 1)

                # scoresT[j, i] for this slot, built 32 rows at a time.
                sc_ps = ps_sc.tile([128, 256], f32, tag="sc")

                def score_mm(j, rhs):
                    sg, jl = divmod(j, 32)
                    nc.tensor.matmul(
                        sc_ps[sg * 32:(sg + 1) * 32, :],
                        wvd_sb[:, jl, :],
                        rhs,
                        start=(jl == 0), stop=(jl == 31),
                        tile_position=(0, sg * 32))

                if CONFIG.get("bias_mode", "tail") == "spread" and BIAS_KEYS:
                    # per 32-key score block: first (32-bpp) keys via DVE
                    # adds + one grouped tanh, last bpp keys via fused
                    # bias-tanh on ACT (spread evenly across the slot)
                    bpp = BIAS_KEYS // 4
                    for blk in range(4):
                        j0 = blk * 32
                        glen = 32 - bpp
                        sums = sums_p.tile([128, ACT_G, 256], bf16,
                                           tag="sums")
                        for jj in range(glen):
                            nc.vector.tensor_scalar_add(
                                out=sums[:, jj, :], in0=qp_sb[:],
                                scalar1=kp_sb[:, j0 + jj:j0 + jj + 1])
                        th = tanh_p.tile([128, ACT_G, 256], bf16, tag="th")
                        nc.scalar.activation(out=th[:, :glen, :],
                                             in_=sums[:, :glen, :],
                                             func=AF.Tanh)
                        for jj in range(glen):
                            score_mm(j0 + jj, th[:, jj, :])
                        thb = tanh_p.tile([128, max(bpp, 1), 256], bf16,
                                          tag="thb")
                        for bk in range(bpp):
                            j = j0 + glen + bk
                            nc.scalar.activation(out=thb[:, bk, :],
                                                 in_=qp_sb[:], func=AF.Tanh,
                                                 bias=kp_sb[:, j:j + 1])
                            score_mm(j, thb[:, bk, :])
                else:
                    for j0, glen in groups:
                        sums = sums_p.tile([128, ACT_G, 256], bf16,
                                           tag="sums")
                        for jj in range(glen):
                            nc.vector.tensor_scalar_add(
                                out=sums[:, jj, :], in0=qp_sb[:],
                                scalar1=kp_sb[:, j0 + jj:j0 + jj + 1])
                        th = tanh_p.tile([128, ACT_G, 256], bf16, tag="th")
                        nc.scalar.activation(out=th[:, :glen, :],
                                             in_=sums[:, :glen, :],
                                             func=AF.Tanh)
                        for jj in range(glen):
                            score_mm(j0 + jj, th[:, jj, :])

                    if BIAS_KEYS:
                        # tail keys: fused tanh(qp+kp_j) on ACT, no DVE add
                        thb = tanh_p.tile([128, BIAS_KEYS, 256], bf16,
                                          tag="thb")
                        for bk in range(BIAS_KEYS):
                            j = SLOT_KEYS - BIAS_KEYS + bk
                            nc.scalar.activation(out=thb[:, bk, :],
                                                 in_=qp_sb[:], func=AF.Tanh,
                                                 bias=kp_sb[:, j:j + 1])
                            score_mm(j, thb[:, bk, :])

                exp_sb = exp_p.tile([128, 256], f32, tag="exp")
                nc.scalar.activation(out=exp_sb[:], in_=sc_ps[:], func=AF.Exp)

                for ic in range(2):
                    o_ps = ps_out.tile([128, VE_W], f32, tag="o")
                    nc.tensor.matmul(o_ps[:],
                                     exp_sb[:, ic * 128:(ic + 1) * 128],
                                     ve_sb[:],
                                     start=True, stop=True)
                    o_sb = exp_p.tile([128, VE_W], f32, tag="o_sb")
                    copy_eng(o_sb[:], o_ps[:])
                    nc.sync.dma_start(out=out[s, ic], in_=o_sb[:])

    nc.compile()
    return nc


def _get_program(cap: int):
    key = (cap, tuple(sorted(CONFIG.items())))
    if key not in _prog_cache:
        _prog_cache[key] = _build_program(cap)
    return _prog_cache[key]


def _chunkT(a2d: np.ndarray, nfree: int) -> np.ndarray:
    """(n, 256) row-major -> (128, 2, n): [p, c, n] = a2d[n, 128c + p]."""
    return np.ascontiguousarray(
        a2d.T.reshape(2, 128, nfree).transpose(1, 0, 2))


def _prepare(Q_batch, K_batch, V_batch, valid_lens, Wq, Wk, Wv):
    Q = np.asarray(Q_batch, np.float32)
    K = np.asarray(K_batch, np.float32)
    V = np.asarray(V_batch, np.float32)
    L = np.asarray(valid_lens).astype(np.int64)
    Wq = np.asarray(Wq, np.float32)
    Wk = np.asarray(Wk, np.float32)
    Wv = np.asarray(Wv, np.float32)

    # Work list: one slot per 128-key block that contains any valid key.
    slots = []
    for b in range(B):
        nblk = max(1, int(-(-int(L[b]) // SLOT_KEYS)))
        nblk = min(nblk, NKV // SLOT_KEYS)
        for blk in range(nblk):
            slots.append((b, blk * SLOT_KEYS))
    cap = -(-len(slots) // NCORES)

    import ml_dtypes
    wqt = _chunkT(Wq, 128)
    wkt = _chunkT(Wk, 128)
    wvd = np.zeros((128, 32, 32), np.float32)
    wvd[:, np.arange(32), np.arange(32)] = Wv[:, None]
    wvd = wvd.astype(ml_dtypes.bfloat16)

    qts = [_chunkT(Q[b], 256) for b in range(B)]

    in_maps = []
    core_slots = []
    for c in range(NCORES):
        items = slots[c * cap:(c + 1) * cap]
        core_slots.append(items)
        kt_arr = np.zeros((cap, 128, 2, 128), np.float32)
        qt_arr = np.zeros((cap, 128, 2, 256), np.float32)
        ve_arr = np.zeros((cap, 128, VE_W), np.float32)
        for si, (b, j0) in enumerate(items):
            kt_arr[si] = _chunkT(K[b, j0:j0 + SLOT_KEYS], SLOT_KEYS)
            qt_arr[si] = qts[b]
            nval = int(np.clip(int(L[b]) - j0, 0, SLOT_KEYS))
            ve_arr[si, :nval, :256] = V[b, j0:j0 + nval]
            ve_arr[si, :nval, DEN_COL] = 1.0
        in_maps.append({
            "kt": kt_arr, "qt": qt_arr, "ve": ve_arr,
            "wqt": wqt, "wkt": wkt, "wvd": wvd,
        })
    return cap, core_slots, in_maps


def _gather(core_slots, results) -> np.ndarray:
    acc = np.zeros((B, NQ, 257), np.float64)
    for c, items in enumerate(core_slots):
        o = results[c]["out"]  # (cap, 2, 128, VE_W)
        for si, (b, _j0) in enumerate(items):
            part = o[si].reshape(NQ, VE_W)[:, :257]
            acc[b] += part
    return (acc[:, :, :256] / acc[:, :, 256:257]).astype(np.float32)


def _install_ntff_hook():
    """Register the axon NTFF profile hook that bass_utils reads via
    antenv.axon_hooks (the shipped antenv stub lacks that module)."""
    import contextlib
    import ctypes
    import sys
    import types

    try:
        from antenv.axon_hooks import get_axon_ntff_profile_hook
        if get_axon_ntff_profile_hook() is not None:
            return
    except ImportError:
        pass

    so_path = "/opt/axon/libaxon_pjrt.so"
    if not os.path.exists(so_path):
        return
    lib = ctypes.CDLL(so_path)
    if not hasattr(lib, "axon_start_nrt_profile"):
        return
    lib.axon_start_nrt_profile.argtypes = [
        ctypes.POINTER(ctypes.c_int64), ctypes.c_size_t]
    lib.axon_start_nrt_profile.restype = ctypes.c_int64
    lib.axon_stop_nrt_profile.argtypes = [ctypes.c_char_p]
    lib.axon_stop_nrt_profile.restype = ctypes.c_int64

    @contextlib.contextmanager
    def _hook(output_dir, device_ids):
        import jax
        jax.devices()
        if device_ids:
            ids = (ctypes.c_int64 * len(device_ids))(*device_ids)
            rc = lib.axon_start_nrt_profile(ids, len(device_ids))
        else:
            rc = lib.axon_start_nrt_profile(None, 0)
        if rc != 0:
            raise RuntimeError(f"axon_start_nrt_profile rc={rc}")
        try:
            yield
        finally:
            n = lib.axon_stop_nrt_profile(str(output_dir).encode())
            print(f"ntff profile: {n} file(s) written to {output_dir}")

    mod = types.ModuleType("antenv.axon_hooks")
    mod.get_axon_ntff_profile_hook = lambda: _hook
    mod.set_axon_ntff_profile_hook = lambda h: None
    sys.modules["antenv.axon_hooks"] = mod
    import antenv
    antenv.axon_hooks = mod


def run(Q_batch, K_batch, V_batch, valid_lens, Wq, Wk, Wv,
        trace: bool = False):
    """Returns (output, exec_time_ns_or_None)."""
    from concourse.bass_utils import run_bass_kernel_spmd

    if trace:
        _install_ntff_hook()

    cap, core_slots, in_maps = _prepare(
        Q_batch, K_batch, V_batch, valid_lens, Wq, Wk, Wv)
    nc = _get_program(cap)

    if os.environ.get("ADD_ATTN_SIM"):
        from concourse.bass_interp import CoreSim
        ncores = int(os.environ.get("ADD_ATTN_SIM_CORES", NCORES))
        results = []
        for c in range(ncores):
            sim = CoreSim(nc)
            for name, arr in in_maps[c].items():
                sim.tensor(name)[:] = arr
            sim.simulate()
            results.append({"out": np.array(sim.tensor("out"))})
        core_slots = core_slots[:ncores]
        return _gather(core_slots, results), None

    res = run_bass_kernel_spmd(nc, in_maps, core_ids=list(range(NCORES)),
                               trace=trace)
    return _gather(core_slots, res.results), res.exec_time_ns


def kernel(Q_batch, K_batch, V_batch, valid_lens, Wq, Wk, Wv):
    out, _ = run(Q_batch, K_batch, V_batch, valid_lens, Wq, Wk, Wv)
    return out
